# revision 8
# baseline (speedup 1.0000x reference)
"""Trainium2 Bass kernel for CSAM channel self-attention module.

Per batch b (one per NeuronCore, B=8 over 8 cores):
    v      = x2[b].reshape(7, D)                 # D = 64*128*128 = 1048576
    E      = v @ v.T                             # [7,7] gram ("energy")
    att    = softmax(rowmax(E) - E, axis=-1)     # == exp(rowmin(E)-E)/Z
    out    = att @ v
    y[b]   = x1[b] * (gamma*out) + x1[b] = x1[b] * (gamma*out + 1)

Layout: d = q*QF + w*F + f  (Q=16, F=512) -> tiles X_w [112, F] with
partition p = 7*q + m (m = channel-attn row index). Each partition line is a
contiguous F*4-byte DRAM run (DMA friendly).

Pass A: stream X tiles (fp32), cast to fp16 cache tiles [113, F] (row 112 is
constant 1.0), PE-transpose [112,128] column chunks -> PSUM [128,112], copy to
SBUF, gram-matmul accumulate E_psum[112,112] (diag 7x7 blocks hold per-q
partial gram).  fp16 is safe: top-2 energy gaps are >100 while fp16 gram error
is ~+-2.

Then: extract + sum diag blocks -> E[7,7], softmax on-chip, build block-diag
weight W[113,112] = 16 copies of (gamma*att)^T + ones row (adds the +1).

Pass B: out_psum[112, F] = W.T @ Xh_w (single LDWEIGHTS), y = out_psum * x1
(DVE), DMA out.  x2 is read exactly once from HBM.
"""

import sys

import numpy as np

try:
    import concourse.bass as bass
except ImportError:  # grading env fallback
    sys.path.insert(0, "/opt/trn_rl_repo")
    import concourse.bass as bass

from contextlib import ExitStack

import concourse.bacc as bacc
import concourse.tile as tile
from concourse import mybir
from concourse.bass_utils import run_bass_kernel_spmd
from concourse.masks import make_identity

F32 = mybir.dt.float32
F16 = mybir.dt.float16

B = 8
NN = 7            # attention dim
Q = 16            # d-runs per channel
P = NN * Q        # 112 partitions of (q, m)
PK = P + 1        # +1 ones row for the fused "+1"
F = 512           # free dim per tile
D_FULL = 64 * 128 * 128   # 1048576
N_CORES = 8
X1_PREFETCH = 16  # x1 tiles prefetched during pass A


def build_nc(d_total=D_FULL, x1_prefetch=X1_PREFETCH):
    assert d_total % (Q * F) == 0
    w_tiles = d_total // (Q * F)      # 128 at full size
    qf = d_total // Q
    cpt = F // 128                    # 128-col transpose chunks per tile (4)
    n_gram_mm = w_tiles * cpt
    x1_prefetch = min(x1_prefetch, w_tiles)

    nc = bacc.Bacc("TRN2", target_bir_lowering=False, debug=False)
    x1 = nc.dram_tensor("x1", [NN, d_total], F32, kind="ExternalInput")
    x2 = nc.dram_tensor("x2", [NN, d_total], F32, kind="ExternalInput")
    gm = nc.dram_tensor("gamma", [1], F32, kind="ExternalInput")
    y = nc.dram_tensor("y", [NN, d_total], F32, kind="ExternalOutput")

    # [q, m, w, f] views; per-w slice is [Q, NN, F] == [112, F] partitions
    x2v = x2[:].rearrange("m (q w f) -> q m w f", q=Q, w=w_tiles, f=F)
    x1v = x1[:].rearrange("m (q w f) -> q m w f", q=Q, w=w_tiles, f=F)
    yv = y[:].rearrange("m (q w f) -> q m w f", q=Q, w=w_tiles, f=F)

    with tile.TileContext(nc) as tc, ExitStack() as ctx:
        consts = ctx.enter_context(tc.tile_pool(name="consts", bufs=1))
        cache = ctx.enter_context(tc.tile_pool(name="cache", bufs=1))
        xs = ctx.enter_context(tc.tile_pool(name="xs", bufs=4))
        x1p = ctx.enter_context(tc.tile_pool(name="x1p", bufs=1))
        x1s = ctx.enter_context(tc.tile_pool(name="x1s", bufs=3))
        tsb = ctx.enter_context(tc.tile_pool(name="tsb", bufs=4))
        ys = ctx.enter_context(tc.tile_pool(name="ys", bufs=3))
        small = ctx.enter_context(tc.tile_pool(name="small", bufs=1))
        tps = ctx.enter_context(tc.tile_pool(name="tps", bufs=3, space="PSUM"))
        eps = ctx.enter_context(tc.tile_pool(name="eps", bufs=1, space="PSUM"))
        ops = ctx.enter_context(tc.tile_pool(name="ops", bufs=3, space="PSUM"))

        ident = consts.tile([P, P], F16)
        make_identity(nc, ident)
        ones = consts.tile([1, F], F16)
        nc.vector.memset(ones[:], 1.0)

        E = eps.tile([P, P], F32)

        xh = [cache.tile([PK, F], F16, name=f"xh{w}", tag=f"xh{w}") for w in range(w_tiles)]
        x1pre = [x1p.tile([P, F], F32, name=f"x1pre{w}", tag=f"x1p{w}") for w in range(x1_prefetch)]

        # ---------------- pass A: stream x2, cast, transpose, gram ----------
        mm = 0
        for w in range(w_tiles):
            xt = xs.tile([P, F], F32)
            nc.sync.dma_start(out=xt[:], in_=x2v[:, :, w, :])
            nc.sync.dma_start(out=xh[w][P:PK, :], in_=ones[:])  # fused "+1" row
            nc.vector.tensor_copy(out=xh[w][0:P, :], in_=xt[:])  # f32 -> f16
            if w < x1_prefetch:
                nc.sync.dma_start(out=x1pre[w][:], in_=x1v[:, :, w, :])
            for c in range(cpt):
                tp = tps.tile([128, P], F16)
                nc.tensor.transpose(tp[:], xh[w][0:P, c * 128:(c + 1) * 128], ident[:])
                tt = tsb.tile([128, P], F16)
                if c % 2 == 0:
                    nc.scalar.copy(tt[:], tp[:])
                else:
                    nc.vector.tensor_copy(out=tt[:], in_=tp[:])
                nc.tensor.matmul(
                    E[:], lhsT=tt[:], rhs=tt[:],
                    start=(mm == 0), stop=(mm == n_gram_mm - 1),
                )
                mm += 1

        # ---------------- energy -> attention -> weights --------------------
        e_sb = small.tile([P, P], F32)
        nc.scalar.copy(e_sb[:], E[:])                  # PSUM -> SBUF
        eb = small.tile([NN, Q, NN], F32)
        for q in range(Q):
            nc.sync.dma_start(
                out=eb[:, q, :],
                in_=e_sb[7 * q:7 * q + 7, 7 * q:7 * q + 7],
            )
        e7 = small.tile([NN, NN], F32)
        nc.vector.tensor_reduce(
            out=e7[:], in_=eb[:].rearrange("p q m -> p m q"),
            axis=mybir.AxisListType.X, op=mybir.AluOpType.add,
        )
        mn = small.tile([NN, 1], F32)
        nc.vector.tensor_reduce(
            out=mn[:], in_=e7[:], axis=mybir.AxisListType.X,
            op=mybir.AluOpType.min,
        )
        ex = small.tile([NN, NN], F32)
        nc.scalar.activation(
            out=ex[:], in_=e7[:], func=mybir.ActivationFunctionType.Exp,
            bias=mn[:], scale=-1.0,
        )                                              # exp(rowmin - E)
        z = small.tile([NN, 1], F32)
        nc.vector.tensor_reduce(
            out=z[:], in_=ex[:], axis=mybir.AxisListType.X,
            op=mybir.AluOpType.add,
        )
        r = small.tile([NN, 1], F32)
        nc.vector.reciprocal(r[:], z[:])
        gsb = small.tile([NN, 1], F32)
        nc.gpsimd.dma_start(
            out=gsb[:],
            in_=bass.AP(tensor=gm[:].tensor, offset=0, ap=[[0, NN], [1, 1]]),
        )
        rg = small.tile([NN, 1], F32)
        nc.vector.tensor_mul(rg[:], r[:], gsb[:])      # gamma / Z_n
        a32 = small.tile([32, 32], F32)
        nc.vector.memset(a32[:], 0.0)
        nc.vector.tensor_scalar_mul(a32[0:NN, 0:NN], ex[:], rg[:])  # gamma*att
        at32 = small.tile([32, 32], F32)
        nc.vector.transpose(at32[:], a32[:])           # (gamma*att)^T
        wt = small.tile([PK, P], F16)
        nc.vector.memset(wt[:], 0.0)
        nc.sync.dma_start(out=wt[P:PK, :], in_=ones[0:1, 0:P])  # ones row -> "+1"
        for q in range(Q):
            nc.gpsimd.dma_start(                       # f32 -> f16 cast dma
                out=wt[7 * q:7 * q + 7, 7 * q:7 * q + 7],
                in_=at32[0:NN, 0:NN],
            )

        # ---------------- pass B: out = W.T @ Xh; y = out * x1 --------------
        for w in range(w_tiles):
            if w < x1_prefetch:
                x1t = x1pre[w]
            else:
                x1t = x1s.tile([P, F], F32)
                nc.sync.dma_start(out=x1t[:], in_=x1v[:, :, w, :])
            op = ops.tile([P, F], F32)
            nc.tensor.matmul(op[:], lhsT=wt[:], rhs=xh[w][:], start=True, stop=True)
            yt = ys.tile([P, F], F32)
            nc.vector.tensor_mul(yt[:], op[:], x1t[:])
            nc.sync.dma_start(out=yv[:, :, w, :], in_=yt[:])

    nc.compile()
    return nc


_NC_CACHE = {}


def _get_nc(d_total=D_FULL):
    if d_total not in _NC_CACHE:
        _NC_CACHE[d_total] = build_nc(d_total)
    return _NC_CACHE[d_total]


def kernel(x1: np.ndarray, x2: np.ndarray, gamma: np.ndarray) -> np.ndarray:
    b, n, c, h, w = x1.shape
    assert (b, n) == (B, NN)
    d = c * h * w
    x1r = np.ascontiguousarray(x1.reshape(b, n, d)).astype(np.float32, copy=False)
    x2r = np.ascontiguousarray(x2.reshape(b, n, d)).astype(np.float32, copy=False)
    g = np.asarray(gamma, dtype=np.float32).reshape(1)

    nc = _get_nc(d)
    in_maps = [
        {"x1": x1r[i], "x2": x2r[i], "gamma": g} for i in range(N_CORES)
    ]
    res = run_bass_kernel_spmd(nc, in_maps, list(range(N_CORES)))
    out = np.stack([res.results[i]["y"] for i in range(N_CORES)], axis=0)
    return out.reshape(b, n, c, h, w).astype(np.float32, copy=False)


# revision 10
# speedup vs baseline: 1.0177x; 1.0177x over previous
"""Trainium2 Bass kernel for CSAM channel self-attention module.

Per batch b (one per NeuronCore, B=8 over 8 cores):
    v      = x2[b].reshape(7, D)                 # D = 64*128*128 = 1048576
    E      = v @ v.T                             # [7,7] gram ("energy")
    att    = softmax(rowmax(E) - E, axis=-1)     # == exp(rowmin(E)-E)/Z
    out    = att @ v
    y[b]   = x1[b] * (gamma*out) + x1[b] = x1[b] * (gamma*out + 1)

Layout: d = q*65536 + w*2048 + f  (Q=16 runs, stream tiles [112, 2048] with
partition p = 7*q + m and 8KB contiguous DRAM lines).

Pass A: stream x2 (alternating the two HWDGE queues: SP + ACT), cast fp32 ->
fp16 cache tiles [113, 2048*2-grouped] (row 112 = 1.0 for the fused "+1"),
PE-transpose [112,128] chunks -> PSUM [128,112] fp16, copy to SBUF (DVE/ACT
alternating), gram-matmul accumulate into E_psum[112,112] (diag 7x7 blocks =
per-q partial gram).  Gram matmuls trail the transposes by a few chunks so
the in-order PE queue never stalls on the copy round-trip.  fp16 is safe:
top-2 energy gaps are >100 while fp16 gram error is ~+-2.

Then: extract + sum diag blocks via a DRAM bounce -> E[7,7], softmax on-chip,
build block-diag W[113,112] = 16 copies of (gamma*att)^T + ones row (DRAM
bounce to keep per-instruction sync fan-in at 1).

Pass B: out_psum[112,512] = W.T @ Xh slices (fp16, weights resident), then
y = out_psum * x1 on DVE, staged into [112,2048] tiles, DMA out on SP while
x1 streams on ACT.  x2 is read from HBM exactly once.
"""

import sys

import numpy as np

try:
    import concourse.bass as bass
except ImportError:  # grading env fallback
    sys.path.insert(0, "/opt/trn_rl_repo")
    import concourse.bass as bass

from contextlib import ExitStack

import concourse.bacc as bacc
import concourse.tile as tile
from concourse import mybir
from concourse.bass_utils import run_bass_kernel_spmd
from concourse.masks import make_identity

F32 = mybir.dt.float32
F16 = mybir.dt.float16

B = 8
NN = 7              # attention dim
Q = 16              # d-runs per channel
P = NN * Q          # 112 partitions of (q, m)
PK = P + 1          # +1 ones row for the fused "+1"
FS = 2048           # stream tile free dim (8KB DRAM lines)
FM = 512            # matmul slice free dim (one PSUM bank)
D_FULL = 64 * 128 * 128
N_CORES = 8
PIPE = 6            # gram matmul trails transposes by this many chunks


def build_nc(d_total=D_FULL):
    assert d_total % (Q * FS) == 0
    ws = d_total // (Q * FS)          # stream tiles (32 at full size)
    cpt = FS // 128                   # transpose chunks per stream tile (16)
    mpt = FS // FM                    # matmul slices per stream tile (4)
    n_gram = ws * cpt

    nc = bacc.Bacc("TRN2", target_bir_lowering=False, debug=False)
    x1 = nc.dram_tensor("x1", [NN, d_total], F32, kind="ExternalInput")
    x2 = nc.dram_tensor("x2", [NN, d_total], F32, kind="ExternalInput")
    gm = nc.dram_tensor("gamma", [1], F32, kind="ExternalInput")
    y = nc.dram_tensor("y", [NN, d_total], F32, kind="ExternalOutput")

    x2v = x2[:].rearrange("m (q w f) -> q m w f", q=Q, w=ws, f=FS)
    x1v = x1[:].rearrange("m (q w f) -> q m w f", q=Q, w=ws, f=FS)
    yv = y[:].rearrange("m (q w f) -> q m w f", q=Q, w=ws, f=FS)

    with tile.TileContext(nc) as tc, ExitStack() as ctx:
        consts = ctx.enter_context(tc.tile_pool(name="consts", bufs=1))
        cache = ctx.enter_context(tc.tile_pool(name="cache", bufs=1))
        xs = ctx.enter_context(tc.tile_pool(name="xs", bufs=2))
        x1s = ctx.enter_context(tc.tile_pool(name="x1s", bufs=2))
        tsb = ctx.enter_context(tc.tile_pool(name="tsb", bufs=PIPE + 3))
        ys = ctx.enter_context(tc.tile_pool(name="ys", bufs=2))
        small = ctx.enter_context(tc.tile_pool(name="small", bufs=1))
        dramp = ctx.enter_context(tc.tile_pool(name="dramp", bufs=1, space="DRAM"))
        tps = ctx.enter_context(tc.tile_pool(name="tps", bufs=4, space="PSUM"))
        eps = ctx.enter_context(tc.tile_pool(name="eps", bufs=1, space="PSUM"))
        ops = ctx.enter_context(tc.tile_pool(name="ops", bufs=3, space="PSUM"))

        ident = consts.tile([P, P], F16)
        make_identity(nc, ident)
        ones = consts.tile([1, FM], F16)
        nc.vector.memset(ones[:], 1.0)
        ones_bc = bass.AP(
            tensor=ones.tensor, offset=ones.offset,
            ap=[list(ones.ap[0]), [0, FS // FM], [1, FM]],
        )

        # W template (zeros + ones row) staged to DRAM early, off critical path
        wscr = dramp.tile([PK, P], F16)
        wtmpl = small.tile([PK, P], F16)
        nc.vector.memset(wtmpl[0:P, :], 0.0)
        nc.sync.dma_start(out=wtmpl[P:PK, :], in_=ones[0:1, 0:P])
        nc.sync.dma_start(out=wscr[:], in_=wtmpl[:])
        escr = dramp.tile([P, P], F32)

        E = eps.tile([P, P], F32)
        xh = [cache.tile([PK, FS], F16, name=f"xh{w}", tag=f"xh{w}")
              for w in range(ws)]

        # ---------------- pass A: stream x2, cast, transpose, gram ----------
        pend = []          # (tt, idx) awaiting gram matmul
        gi = 0             # gram matmuls emitted

        def emit_gram(tt):
            nonlocal gi
            nc.tensor.matmul(E[:], lhsT=tt[:], rhs=tt[:],
                             start=(gi == 0), stop=(gi == n_gram - 1))
            gi += 1

        for w in range(ws):
            xt = xs.tile([P, FS], F32)
            dmae = nc.sync if w % 2 == 0 else nc.scalar
            dmae.dma_start(out=xt[:], in_=x2v[:, :, w, :])
            nc.sync.dma_start(out=xh[w][P:PK, :], in_=ones_bc)  # "+1" row
            nc.gpsimd.tensor_copy(out=xh[w][0:P, :], in_=xt[:])  # f32 -> f16
            for c in range(cpt):
                tp = tps.tile([128, P], F16)
                nc.tensor.transpose(
                    tp[:], xh[w][0:P, c * 128:(c + 1) * 128], ident[:])
                tt = tsb.tile([128, P], F16)
                if c % 2 == 0:
                    nc.scalar.copy(tt[:], tp[:])
                else:
                    nc.vector.tensor_copy(out=tt[:], in_=tp[:])
                pend.append(tt)
                if len(pend) > PIPE:
                    emit_gram(pend.pop(0))
        for tt in pend:
            emit_gram(tt)
        pend = []

        # ---------------- energy -> attention -> weights --------------------
        e_sb = small.tile([P, P], F32)
        nc.scalar.copy(e_sb[:], E[:])                  # PSUM -> SBUF
        nc.sync.dma_start(out=escr[:], in_=e_sb[:])    # bounce via DRAM
        eb = small.tile([NN, Q, NN], F32)
        # gather the 16 diagonal 7x7 blocks in one DMA
        diag_src = bass.AP(
            tensor=escr.tensor, offset=escr.offset,
            ap=[[P, NN], [NN * P + NN, Q], [1, NN]],
        )
        nc.sync.dma_start(out=eb[:], in_=diag_src)
        e7 = small.tile([NN, NN], F32)
        nc.vector.tensor_reduce(
            out=e7[:], in_=eb[:].rearrange("p q m -> p m q"),
            axis=mybir.AxisListType.X, op=mybir.AluOpType.add,
        )
        mn = small.tile([NN, 1], F32)
        nc.vector.tensor_reduce(
            out=mn[:], in_=e7[:], axis=mybir.AxisListType.X,
            op=mybir.AluOpType.min,
        )
        ex = small.tile([NN, NN], F32)
        nc.scalar.activation(
            out=ex[:], in_=e7[:], func=mybir.ActivationFunctionType.Exp,
            bias=mn[:], scale=-1.0,
        )                                              # exp(rowmin - E)
        z = small.tile([NN, 1], F32)
        nc.vector.tensor_reduce(
            out=z[:], in_=ex[:], axis=mybir.AxisListType.X,
            op=mybir.AluOpType.add,
        )
        r = small.tile([NN, 1], F32)
        nc.vector.reciprocal(r[:], z[:])
        gsb = small.tile([NN, 1], F32)
        nc.gpsimd.dma_start(
            out=gsb[:],
            in_=bass.AP(tensor=gm[:].tensor, offset=0, ap=[[0, NN], [1, 1]]),
        )
        rg = small.tile([NN, 1], F32)
        nc.vector.tensor_mul(rg[:], r[:], gsb[:])      # gamma / Z_n
        a32 = small.tile([32, 32], F32)
        nc.vector.memset(a32[:], 0.0)
        nc.vector.tensor_scalar_mul(a32[0:NN, 0:NN], ex[:], rg[:])  # gamma*att
        at32 = small.tile([32, 32], F32)
        nc.vector.transpose(at32[:], a32[:])           # (gamma*att)^T
        # scatter the 16 diag blocks of W via DRAM (src bcast over q, f32->f16)
        # dims ordered (m, q, n) on both sides; src partition dim must be first
        diag_dst = bass.AP(
            tensor=wscr.tensor, offset=wscr.offset,
            ap=[[P, NN], [NN * P + NN, Q], [1, NN]],
        )
        at_src = bass.AP(
            tensor=at32.tensor, offset=at32.offset,
            ap=[[at32.ap[0][0], NN], [0, Q], [1, NN]],
        )
        nc.gpsimd.dma_start(out=diag_dst, in_=at_src)  # casts f32 -> f16
        wt = small.tile([PK, P], F16)
        nc.sync.dma_start(out=wt[:], in_=wscr[:])

        # ---------------- pass B: out = W.T @ Xh; y = out * x1 --------------
        for w in range(ws):
            x1t = x1s.tile([P, FS], F32)
            nc.scalar.dma_start(out=x1t[:], in_=x1v[:, :, w, :])
            yt = ys.tile([P, FS], F32)
            for j in range(mpt):
                sl = slice(j * FM, (j + 1) * FM)
                op = ops.tile([P, FM], F32)
                nc.tensor.matmul(op[:], lhsT=wt[:], rhs=xh[w][:, sl],
                                 start=True, stop=True)
                nc.vector.tensor_mul(yt[:, sl], op[:], x1t[:, sl])
            nc.sync.dma_start(out=yv[:, :, w, :], in_=yt[:])

    nc.compile()
    return nc


_NC_CACHE = {}


def _get_nc(d_total=D_FULL):
    if d_total not in _NC_CACHE:
        _NC_CACHE[d_total] = build_nc(d_total)
    return _NC_CACHE[d_total]


def kernel(x1: np.ndarray, x2: np.ndarray, gamma: np.ndarray) -> np.ndarray:
    b, n, c, h, w = x1.shape
    assert (b, n) == (B, NN)
    d = c * h * w
    x1r = np.ascontiguousarray(x1.reshape(b, n, d)).astype(np.float32, copy=False)
    x2r = np.ascontiguousarray(x2.reshape(b, n, d)).astype(np.float32, copy=False)
    g = np.asarray(gamma, dtype=np.float32).reshape(1)

    nc = _get_nc(d)
    in_maps = [
        {"x1": x1r[i], "x2": x2r[i], "gamma": g} for i in range(N_CORES)
    ]
    res = run_bass_kernel_spmd(nc, in_maps, list(range(N_CORES)))
    out = np.stack([res.results[i]["y"] for i in range(N_CORES)], axis=0)
    return out.reshape(b, n, c, h, w).astype(np.float32, copy=False)


# revision 12
# speedup vs baseline: 1.2752x; 1.2530x over previous
"""Trainium2 Bass kernel for CSAM channel self-attention module.

Per batch b (one per NeuronCore, B=8 over 8 cores):
    v      = x2[b].reshape(7, D)                 # D = 64*128*128 = 1048576
    E      = v @ v.T                             # [7,7] gram ("energy")
    att    = softmax(rowmax(E) - E, axis=-1)     # == exp(rowmin(E)-E)/Z
    out    = att @ v
    y[b]   = x1[b] * (gamma*out) + x1[b] = x1[b] * (gamma*out + 1)

Layout: d = q*65536 + w*2048 + f  (Q=16 runs, stream tiles [112, 2048] with
partition p = 7*q + m and 8KB contiguous DRAM lines).

Pass A: stream x2 (alternating the two HWDGE queues: SP + ACT), cast fp32 ->
fp16 cache tiles [113, 2048*2-grouped] (row 112 = 1.0 for the fused "+1"),
PE-transpose [112,128] chunks -> PSUM [128,112] fp16, copy to SBUF (DVE/ACT
alternating), gram-matmul accumulate into E_psum[112,112] (diag 7x7 blocks =
per-q partial gram).  Gram matmuls trail the transposes by a few chunks so
the in-order PE queue never stalls on the copy round-trip.  fp16 is safe:
top-2 energy gaps are >100 while fp16 gram error is ~+-2.

Then: extract + sum diag blocks via a DRAM bounce -> E[7,7], softmax on-chip,
build block-diag W[113,112] = 16 copies of (gamma*att)^T + ones row (DRAM
bounce to keep per-instruction sync fan-in at 1).

Pass B: out_psum[112,512] = W.T @ Xh slices (fp16, weights resident), then
y = out_psum * x1 on DVE, staged into [112,2048] tiles, DMA out on SP while
x1 streams on ACT.  x2 is read from HBM exactly once.
"""

import sys

import numpy as np

try:
    import concourse.bass as bass
except ImportError:  # grading env fallback
    sys.path.insert(0, "/opt/trn_rl_repo")
    import concourse.bass as bass

from contextlib import ExitStack

import concourse.bacc as bacc
import concourse.tile as tile
from concourse import mybir
from concourse.bass_utils import run_bass_kernel_spmd
from concourse.masks import make_identity

F32 = mybir.dt.float32
F16 = mybir.dt.float16

B = 8
NN = 7              # attention dim
Q = 16              # d-runs per channel
P = NN * Q          # 112 partitions of (q, m)
PK = P + 1          # +1 ones row for the fused "+1"
FS = 2048           # stream tile free dim (8KB DRAM lines)
FM = 512            # matmul slice free dim (one PSUM bank)
D_FULL = 64 * 128 * 128
N_CORES = 8
PIPE = 8            # gram matmul trails transposes by this many chunks


def build_nc(d_total=D_FULL):
    assert d_total % (Q * FS) == 0
    ws = d_total // (Q * FS)          # stream tiles (32 at full size)
    cpt = FS // 128                   # transpose chunks per stream tile (16)
    mpt = FS // FM                    # matmul slices per stream tile (4)
    n_gram = ws * cpt

    nc = bacc.Bacc("TRN2", target_bir_lowering=False, debug=False)
    x1 = nc.dram_tensor("x1", [NN, d_total], F32, kind="ExternalInput")
    x2 = nc.dram_tensor("x2", [NN, d_total], F32, kind="ExternalInput")
    gm = nc.dram_tensor("gamma", [1], F32, kind="ExternalInput")
    y = nc.dram_tensor("y", [NN, d_total], F32, kind="ExternalOutput")

    x2v = x2[:].rearrange("m (q w f) -> q m w f", q=Q, w=ws, f=FS)
    x1v = x1[:].rearrange("m (q w f) -> q m w f", q=Q, w=ws, f=FS)
    yv = y[:].rearrange("m (q w f) -> q m w f", q=Q, w=ws, f=FS)

    with tile.TileContext(nc) as tc, ExitStack() as ctx:
        consts = ctx.enter_context(tc.tile_pool(name="consts", bufs=1))
        cache = ctx.enter_context(tc.tile_pool(name="cache", bufs=1))
        xs = ctx.enter_context(tc.tile_pool(name="xs", bufs=2))
        x1s = ctx.enter_context(tc.tile_pool(name="x1s", bufs=2))
        tsb = ctx.enter_context(tc.tile_pool(name="tsb", bufs=PIPE + 3))
        ys = ctx.enter_context(tc.tile_pool(name="ys", bufs=2))
        small = ctx.enter_context(tc.tile_pool(name="small", bufs=1))
        dramp = ctx.enter_context(tc.tile_pool(name="dramp", bufs=1, space="DRAM"))
        tps = ctx.enter_context(tc.tile_pool(name="tps", bufs=4, space="PSUM"))
        eps = ctx.enter_context(tc.tile_pool(name="eps", bufs=1, space="PSUM"))
        ops = ctx.enter_context(tc.tile_pool(name="ops", bufs=3, space="PSUM"))

        ident = consts.tile([P, P], F16)
        make_identity(nc, ident)
        ones = consts.tile([1, FM], F16)
        nc.vector.memset(ones[:], 1.0)
        ones_bc = bass.AP(
            tensor=ones.tensor, offset=ones.offset,
            ap=[list(ones.ap[0]), [0, FS // FM], [1, FM]],
        )

        # W template (zeros + ones row) staged to DRAM early, off critical path
        wscr = dramp.tile([PK, P], F16)
        wtmpl = small.tile([PK, P], F16)
        nc.vector.memset(wtmpl[0:P, :], 0.0)
        nc.sync.dma_start(out=wtmpl[P:PK, :], in_=ones[0:1, 0:P])
        nc.sync.dma_start(out=wscr[:], in_=wtmpl[:])
        escr = dramp.tile([P, P], F32)

        E = eps.tile([P, P], F32)
        xh = [cache.tile([PK, FS], F16, name=f"xh{w}", tag=f"xh{w}")
              for w in range(ws)]

        # ---------------- pass A: stream x2, cast, transpose, gram ----------
        pend = []          # (tt, idx) awaiting gram matmul
        gi = 0             # gram matmuls emitted

        def emit_gram(tt):
            nonlocal gi
            nc.tensor.matmul(E[:], lhsT=tt[:], rhs=tt[:],
                             start=(gi == 0), stop=(gi == n_gram - 1))
            gi += 1

        for w in range(ws):
            xt = xs.tile([P, FS], F32)
            dmae = nc.sync if w % 2 == 0 else nc.scalar
            dmae.dma_start(out=xt[:], in_=x2v[:, :, w, :])
            nc.sync.dma_start(out=xh[w][P:PK, :], in_=ones_bc)  # "+1" row
            if w % 2 == 0:
                nc.vector.tensor_copy(out=xh[w][0:P, :], in_=xt[:])  # f32->f16
            else:
                nc.scalar.copy(xh[w][0:P, :], xt[:])
            for c in range(cpt):
                tp = tps.tile([128, P], F16)
                nc.tensor.transpose(
                    tp[:], xh[w][0:P, c * 128:(c + 1) * 128], ident[:])
                tt = tsb.tile([128, P], F16)
                if c % 2 == 0:
                    nc.scalar.copy(tt[:], tp[:])
                else:
                    nc.vector.tensor_copy(out=tt[:], in_=tp[:])
                pend.append(tt)
                if len(pend) > PIPE:
                    emit_gram(pend.pop(0))
        for tt in pend:
            emit_gram(tt)
        pend = []

        # ---------------- energy -> attention -> weights --------------------
        e_sb = small.tile([P, P], F32)
        nc.scalar.copy(e_sb[:], E[:])                  # PSUM -> SBUF
        nc.sync.dma_start(out=escr[:], in_=e_sb[:])    # bounce via DRAM
        eb = small.tile([NN, Q, NN], F32)
        # gather the 16 diagonal 7x7 blocks in one DMA
        diag_src = bass.AP(
            tensor=escr.tensor, offset=escr.offset,
            ap=[[P, NN], [NN * P + NN, Q], [1, NN]],
        )
        nc.sync.dma_start(out=eb[:], in_=diag_src)
        e7 = small.tile([NN, NN], F32)
        nc.vector.tensor_reduce(
            out=e7[:], in_=eb[:].rearrange("p q m -> p m q"),
            axis=mybir.AxisListType.X, op=mybir.AluOpType.add,
        )
        mn = small.tile([NN, 1], F32)
        nc.vector.tensor_reduce(
            out=mn[:], in_=e7[:], axis=mybir.AxisListType.X,
            op=mybir.AluOpType.min,
        )
        ex = small.tile([NN, NN], F32)
        nc.scalar.activation(
            out=ex[:], in_=e7[:], func=mybir.ActivationFunctionType.Exp,
            bias=mn[:], scale=-1.0,
        )                                              # exp(rowmin - E)
        z = small.tile([NN, 1], F32)
        nc.vector.tensor_reduce(
            out=z[:], in_=ex[:], axis=mybir.AxisListType.X,
            op=mybir.AluOpType.add,
        )
        r = small.tile([NN, 1], F32)
        nc.vector.reciprocal(r[:], z[:])
        gsb = small.tile([NN, 1], F32)
        nc.gpsimd.dma_start(
            out=gsb[:],
            in_=bass.AP(tensor=gm[:].tensor, offset=0, ap=[[0, NN], [1, 1]]),
        )
        rg = small.tile([NN, 1], F32)
        nc.vector.tensor_mul(rg[:], r[:], gsb[:])      # gamma / Z_n
        a32 = small.tile([32, 32], F32)
        nc.vector.memset(a32[:], 0.0)
        nc.vector.tensor_scalar_mul(a32[0:NN, 0:NN], ex[:], rg[:])  # gamma*att
        at32 = small.tile([32, 32], F32)
        nc.vector.transpose(at32[:], a32[:])           # (gamma*att)^T
        # scatter the 16 diag blocks of W via DRAM (src bcast over q, f32->f16)
        # dims ordered (m, q, n) on both sides; src partition dim must be first
        diag_dst = bass.AP(
            tensor=wscr.tensor, offset=wscr.offset,
            ap=[[P, NN], [NN * P + NN, Q], [1, NN]],
        )
        at_src = bass.AP(
            tensor=at32.tensor, offset=at32.offset,
            ap=[[at32.ap[0][0], NN], [0, Q], [1, NN]],
        )
        nc.gpsimd.dma_start(out=diag_dst, in_=at_src)  # casts f32 -> f16
        wt = small.tile([PK, P], F16)
        nc.sync.dma_start(out=wt[:], in_=wscr[:])

        # ---------------- pass B: out = W.T @ Xh; y = out * x1 --------------
        for w in range(ws):
            x1t = x1s.tile([P, FS], F32)
            nc.scalar.dma_start(out=x1t[:], in_=x1v[:, :, w, :])
            yt = ys.tile([P, FS], F32)
            for j in range(mpt):
                sl = slice(j * FM, (j + 1) * FM)
                op = ops.tile([P, FM], F32)
                nc.tensor.matmul(op[:], lhsT=wt[:], rhs=xh[w][:, sl],
                                 start=True, stop=True)
                nc.vector.tensor_mul(yt[:, sl], op[:], x1t[:, sl])
            nc.sync.dma_start(out=yv[:, :, w, :], in_=yt[:])

    nc.compile()
    return nc


_NC_CACHE = {}


def _get_nc(d_total=D_FULL):
    if d_total not in _NC_CACHE:
        _NC_CACHE[d_total] = build_nc(d_total)
    return _NC_CACHE[d_total]


def kernel(x1: np.ndarray, x2: np.ndarray, gamma: np.ndarray) -> np.ndarray:
    b, n, c, h, w = x1.shape
    assert (b, n) == (B, NN)
    d = c * h * w
    x1r = np.ascontiguousarray(x1.reshape(b, n, d)).astype(np.float32, copy=False)
    x2r = np.ascontiguousarray(x2.reshape(b, n, d)).astype(np.float32, copy=False)
    g = np.asarray(gamma, dtype=np.float32).reshape(1)

    nc = _get_nc(d)
    in_maps = [
        {"x1": x1r[i], "x2": x2r[i], "gamma": g} for i in range(N_CORES)
    ]
    res = run_bass_kernel_spmd(nc, in_maps, list(range(N_CORES)))
    out = np.stack([res.results[i]["y"] for i in range(N_CORES)], axis=0)
    return out.reshape(b, n, c, h, w).astype(np.float32, copy=False)


# revision 16
# speedup vs baseline: 1.2760x; 1.0006x over previous
"""Trainium2 Bass kernel for CSAM channel self-attention module.

Per batch b (one per NeuronCore, B=8 over 8 cores):
    v      = x2[b].reshape(7, D)                 # D = 64*128*128 = 1048576
    E      = v @ v.T                             # [7,7] gram ("energy")
    att    = softmax(rowmax(E) - E, axis=-1)     # == exp(rowmin(E)-E)/Z
    out    = att @ v
    y[b]   = x1[b] * (gamma*out) + x1[b] = x1[b] * (gamma*out + 1)

Layout: d = q*65536 + w*2048 + f  (Q=16 runs, stream tiles [112, 2048] with
partition p = 7*q + m and 8KB contiguous DRAM lines).

Pass A: stream x2 (alternating the two HWDGE queues: SP + ACT), cast fp32 ->
fp16 cache tiles [113, 2048*2-grouped] (row 112 = 1.0 for the fused "+1"),
PE-transpose [112,128] chunks -> PSUM [128,112] fp16, copy to SBUF (DVE/ACT
alternating), gram-matmul accumulate into E_psum[112,112] (diag 7x7 blocks =
per-q partial gram).  Gram matmuls trail the transposes by a few chunks so
the in-order PE queue never stalls on the copy round-trip.  fp16 is safe:
top-2 energy gaps are >100 while fp16 gram error is ~+-2.

Then: extract + sum diag blocks via a DRAM bounce -> E[7,7], softmax on-chip,
build block-diag W[113,112] = 16 copies of (gamma*att)^T + ones row (DRAM
bounce to keep per-instruction sync fan-in at 1).

Pass B: out_psum[112,512] = W.T @ Xh slices (fp16, weights resident), then
y = out_psum * x1 on DVE, staged into [112,2048] tiles, DMA out on SP while
x1 streams on ACT.  x2 is read from HBM exactly once.
"""

import sys

import numpy as np

try:
    import concourse.bass as bass
except ImportError:  # grading env fallback
    sys.path.insert(0, "/opt/trn_rl_repo")
    import concourse.bass as bass

from contextlib import ExitStack

import concourse.bacc as bacc
import concourse.tile as tile
from concourse import mybir
from concourse.bass_utils import run_bass_kernel_spmd
from concourse.masks import make_identity

F32 = mybir.dt.float32
F16 = mybir.dt.float16

B = 8
NN = 7              # attention dim
Q = 16              # d-runs per channel
P = NN * Q          # 112 partitions of (q, m)
PK = P + 1          # +1 ones row for the fused "+1"
FS = 2048           # stream tile free dim (8KB DRAM lines)
FM = 512            # matmul slice free dim (one PSUM bank)
D_FULL = 64 * 128 * 128
N_CORES = 8
PIPE = 8            # gram matmul trails transposes by this many chunks


def build_nc(d_total=D_FULL):
    assert d_total % (Q * FS) == 0
    ws = d_total // (Q * FS)          # stream tiles (32 at full size)
    cpt = FS // 128                   # transpose chunks per stream tile (16)
    mpt = FS // FM                    # matmul slices per stream tile (4)
    n_gram = ws * cpt

    nc = bacc.Bacc("TRN2", target_bir_lowering=False, debug=False)
    x1 = nc.dram_tensor("x1", [NN, d_total], F32, kind="ExternalInput")
    x2 = nc.dram_tensor("x2", [NN, d_total], F32, kind="ExternalInput")
    gm = nc.dram_tensor("gamma", [1], F32, kind="ExternalInput")
    y = nc.dram_tensor("y", [NN, d_total], F32, kind="ExternalOutput")

    x2v = x2[:].rearrange("m (q w f) -> q m w f", q=Q, w=ws, f=FS)
    x1v = x1[:].rearrange("m (q w f) -> q m w f", q=Q, w=ws, f=FS)
    yv = y[:].rearrange("m (q w f) -> q m w f", q=Q, w=ws, f=FS)

    with tile.TileContext(nc) as tc, ExitStack() as ctx:
        consts = ctx.enter_context(tc.tile_pool(name="consts", bufs=1))
        cache = ctx.enter_context(tc.tile_pool(name="cache", bufs=1))
        xs = ctx.enter_context(tc.tile_pool(name="xs", bufs=2))
        x1s = ctx.enter_context(tc.tile_pool(name="x1s", bufs=2))
        tsb = ctx.enter_context(tc.tile_pool(name="tsb", bufs=PIPE + 3))
        ys = ctx.enter_context(tc.tile_pool(name="ys", bufs=2))
        small = ctx.enter_context(tc.tile_pool(name="small", bufs=1))
        dramp = ctx.enter_context(tc.tile_pool(name="dramp", bufs=1, space="DRAM"))
        tps = ctx.enter_context(tc.tile_pool(name="tps", bufs=5, space="PSUM"))
        eps = ctx.enter_context(tc.tile_pool(name="eps", bufs=1, space="PSUM"))
        ops = ctx.enter_context(tc.tile_pool(name="ops", bufs=2, space="PSUM"))

        ident = consts.tile([P, P], F16)
        make_identity(nc, ident)
        ones = consts.tile([1, FM], F16)
        nc.vector.memset(ones[:], 1.0)
        ones_bc = bass.AP(
            tensor=ones.tensor, offset=ones.offset,
            ap=[list(ones.ap[0]), [0, FS // FM], [1, FM]],
        )

        # W template (zeros + ones row) staged to DRAM early, off critical path
        wscr = dramp.tile([PK, P], F16)
        wtmpl = small.tile([PK, P], F16)
        nc.vector.memset(wtmpl[0:P, :], 0.0)
        nc.sync.dma_start(out=wtmpl[P:PK, :], in_=ones[0:1, 0:P])
        nc.sync.dma_start(out=wscr[:], in_=wtmpl[:])
        escr = dramp.tile([P, P], F32)

        E = eps.tile([P, P], F32)
        xh = [cache.tile([PK, FS], F16, name=f"xh{w}", tag=f"xh{w}")
              for w in range(ws)]

        # ~5.6us of dummy matmuls so the PE HAM clock-gate opens before the
        # real pass-A stream arrives (and stays open)
        for _ in range(60):
            wm = ops.tile([P, P], F32, tag="op")
            nc.tensor.matmul(wm[:], lhsT=ident[:], rhs=ident[:],
                             start=True, stop=True)

        # ---------------- pass A: stream x2, cast, transpose, gram ----------
        pend = []          # (tt, idx) awaiting gram matmul
        gi = 0             # gram matmuls emitted

        def emit_gram(tt):
            nonlocal gi
            nc.tensor.matmul(E[:], lhsT=tt[:], rhs=tt[:],
                             start=(gi == 0), stop=(gi == n_gram - 1))
            gi += 1

        for w in range(ws):
            xt = xs.tile([P, FS], F32)
            dmae = nc.sync if w % 2 == 0 else nc.scalar
            dmae.dma_start(out=xt[:], in_=x2v[:, :, w, :])
            nc.sync.dma_start(out=xh[w][P:PK, :], in_=ones_bc)  # "+1" row
            if w % 2 == 0:
                nc.vector.tensor_copy(out=xh[w][0:P, :], in_=xt[:])  # f32->f16
            else:
                nc.scalar.copy(xh[w][0:P, :], xt[:])
            for c in range(cpt):
                tp = tps.tile([128, P], F16)
                nc.tensor.transpose(
                    tp[:], xh[w][0:P, c * 128:(c + 1) * 128], ident[:])
                tt = tsb.tile([128, P], F16)
                if c % 3 == 0:
                    nc.scalar.copy(tt[:], tp[:])
                else:
                    nc.vector.tensor_copy(out=tt[:], in_=tp[:])
                pend.append(tt)
                if len(pend) > PIPE:
                    emit_gram(pend.pop(0))
        for tt in pend:
            emit_gram(tt)
        pend = []

        # ---------------- energy -> attention -> weights --------------------
        e_sb = small.tile([P, P], F32)
        nc.scalar.copy(e_sb[:], E[:])                  # PSUM -> SBUF
        nc.sync.dma_start(out=escr[:], in_=e_sb[:])    # bounce via DRAM
        eb = small.tile([NN, Q, NN], F32)
        # gather the 16 diagonal 7x7 blocks in one DMA
        diag_src = bass.AP(
            tensor=escr.tensor, offset=escr.offset,
            ap=[[P, NN], [NN * P + NN, Q], [1, NN]],
        )
        nc.sync.dma_start(out=eb[:], in_=diag_src)
        e7 = small.tile([NN, NN], F32)
        nc.vector.tensor_reduce(
            out=e7[:], in_=eb[:].rearrange("p q m -> p m q"),
            axis=mybir.AxisListType.X, op=mybir.AluOpType.add,
        )
        mn = small.tile([NN, 1], F32)
        nc.vector.tensor_reduce(
            out=mn[:], in_=e7[:], axis=mybir.AxisListType.X,
            op=mybir.AluOpType.min,
        )
        ex = small.tile([NN, NN], F32)
        nc.scalar.activation(
            out=ex[:], in_=e7[:], func=mybir.ActivationFunctionType.Exp,
            bias=mn[:], scale=-1.0,
        )                                              # exp(rowmin - E)
        z = small.tile([NN, 1], F32)
        nc.vector.tensor_reduce(
            out=z[:], in_=ex[:], axis=mybir.AxisListType.X,
            op=mybir.AluOpType.add,
        )
        r = small.tile([NN, 1], F32)
        nc.vector.reciprocal(r[:], z[:])
        gsb = small.tile([NN, 1], F32)
        nc.gpsimd.dma_start(
            out=gsb[:],
            in_=bass.AP(tensor=gm[:].tensor, offset=0, ap=[[0, NN], [1, 1]]),
        )
        rg = small.tile([NN, 1], F32)
        nc.vector.tensor_mul(rg[:], r[:], gsb[:])      # gamma / Z_n
        a32 = small.tile([32, 32], F32)
        nc.vector.memset(a32[:], 0.0)
        nc.vector.tensor_scalar_mul(a32[0:NN, 0:NN], ex[:], rg[:])  # gamma*att
        at32 = small.tile([32, 32], F32)
        nc.vector.transpose(at32[:], a32[:])           # (gamma*att)^T
        # scatter the 16 diag blocks of W via DRAM (src bcast over q, f32->f16)
        # dims ordered (m, q, n) on both sides; src partition dim must be first
        diag_dst = bass.AP(
            tensor=wscr.tensor, offset=wscr.offset,
            ap=[[P, NN], [NN * P + NN, Q], [1, NN]],
        )
        at_src = bass.AP(
            tensor=at32.tensor, offset=at32.offset,
            ap=[[at32.ap[0][0], NN], [0, Q], [1, NN]],
        )
        nc.gpsimd.dma_start(out=diag_dst, in_=at_src)  # casts f32 -> f16
        wt = small.tile([PK, P], F16)
        nc.sync.dma_start(out=wt[:], in_=wscr[:])

        # ---------------- pass B: out = W.T @ Xh; y = out * x1 --------------
        for w in range(ws):
            x1t = x1s.tile([P, FS], F32)
            nc.scalar.dma_start(out=x1t[:], in_=x1v[:, :, w, :])
            yt = ys.tile([P, FS], F32)
            for j in range(mpt):
                sl = slice(j * FM, (j + 1) * FM)
                op = ops.tile([P, FM], F32, tag="op")
                nc.tensor.matmul(op[:], lhsT=wt[:], rhs=xh[w][:, sl],
                                 start=True, stop=True)
                nc.vector.tensor_mul(yt[:, sl], op[:], x1t[:, sl])
            nc.gpsimd.dma_start(out=yv[:, :, w, :], in_=yt[:])

    nc.compile()
    return nc


_NC_CACHE = {}


def _get_nc(d_total=D_FULL):
    if d_total not in _NC_CACHE:
        _NC_CACHE[d_total] = build_nc(d_total)
    return _NC_CACHE[d_total]


def kernel(x1: np.ndarray, x2: np.ndarray, gamma: np.ndarray) -> np.ndarray:
    b, n, c, h, w = x1.shape
    assert (b, n) == (B, NN)
    d = c * h * w
    x1r = np.ascontiguousarray(x1.reshape(b, n, d)).astype(np.float32, copy=False)
    x2r = np.ascontiguousarray(x2.reshape(b, n, d)).astype(np.float32, copy=False)
    g = np.asarray(gamma, dtype=np.float32).reshape(1)

    nc = _get_nc(d)
    in_maps = [
        {"x1": x1r[i], "x2": x2r[i], "gamma": g} for i in range(N_CORES)
    ]
    res = run_bass_kernel_spmd(nc, in_maps, list(range(N_CORES)))
    out = np.stack([res.results[i]["y"] for i in range(N_CORES)], axis=0)
    return out.reshape(b, n, c, h, w).astype(np.float32, copy=False)


# revision 21
# speedup vs baseline: 1.4734x; 1.1547x over previous
"""Trainium2 Bass kernel for CSAM channel self-attention module.

Per batch b (one per NeuronCore, B=8 over 8 cores):
    v      = x2[b].reshape(7, D)                 # D = 64*128*128 = 1048576
    E      = v @ v.T                             # [7,7] gram ("energy")
    att    = softmax(rowmax(E) - E, axis=-1)     # == exp(rowmin(E)-E)/Z
    out    = att @ v
    y[b]   = x1[b] * (gamma*out) + x1[b] = x1[b] * (gamma*out + 1)

Layout: d = q*65536 + w*2048 + f  (Q=16 runs, stream tiles [112, 2048] with
partition p = 7*q + m and 8KB contiguous DRAM lines).

Pass A: stream x2 (alternating the two HWDGE queues: SP + ACT), cast fp32 ->
fp16 cache tiles [113, 2048*2-grouped] (row 112 = 1.0 for the fused "+1"),
PE-transpose [112,128] chunks -> PSUM [128,112] fp16, copy to SBUF (DVE/ACT
alternating), gram-matmul accumulate into E_psum[112,112] (diag 7x7 blocks =
per-q partial gram).  Gram matmuls trail the transposes by a few chunks so
the in-order PE queue never stalls on the copy round-trip.  fp16 is safe:
top-2 energy gaps are >100 while fp16 gram error is ~+-2.

Then: extract + sum diag blocks via a DRAM bounce -> E[7,7], softmax on-chip,
build block-diag W[113,112] = 16 copies of (gamma*att)^T + ones row (DRAM
bounce to keep per-instruction sync fan-in at 1).

Pass B: out_psum[112,512] = W.T @ Xh slices (fp16, weights resident), then
y = out_psum * x1 on DVE, staged into [112,2048] tiles, DMA out on SP while
x1 streams on ACT.  x2 is read from HBM exactly once.
"""

import sys

import numpy as np

try:
    import concourse.bass as bass
except ImportError:  # grading env fallback
    sys.path.insert(0, "/opt/trn_rl_repo")
    import concourse.bass as bass

from contextlib import ExitStack

import concourse.bacc as bacc
import concourse.tile as tile
from concourse import mybir
from concourse.bass_utils import run_bass_kernel_spmd
from concourse.masks import make_identity

F32 = mybir.dt.float32
F16 = mybir.dt.float16

B = 8
NN = 7              # attention dim
Q = 16              # d-runs per channel
P = NN * Q          # 112 partitions of (q, m)
PK = P + 1          # +1 ones row for the fused "+1"
FS = 2048           # stream tile free dim (8KB DRAM lines)
FM = 512            # matmul slice free dim (one PSUM bank)
D_FULL = 64 * 128 * 128
N_CORES = 8
PIPE = 8            # gram matmul trails transposes by this many chunks


def build_nc(d_total=D_FULL):
    assert d_total % (Q * FS) == 0
    ws = d_total // (Q * FS)          # stream tiles (32 at full size)
    cpt = FS // 128                   # transpose chunks per stream tile (16)
    mpt = FS // FM                    # matmul slices per stream tile (4)
    n_gram = ws * cpt

    nc = bacc.Bacc("TRN2", target_bir_lowering=False, debug=False)
    x1 = nc.dram_tensor("x1", [NN, d_total], F32, kind="ExternalInput")
    x2 = nc.dram_tensor("x2", [NN, d_total], F32, kind="ExternalInput")
    gm = nc.dram_tensor("gamma", [1], F32, kind="ExternalInput")
    y = nc.dram_tensor("y", [NN, d_total], F32, kind="ExternalOutput")

    x2v = x2[:].rearrange("m (q w f) -> q m w f", q=Q, w=ws, f=FS)
    x1v = x1[:].rearrange("m (q w f) -> q m w f", q=Q, w=ws, f=FS)
    yv = y[:].rearrange("m (q w f) -> q m w f", q=Q, w=ws, f=FS)

    with tile.TileContext(nc) as tc, ExitStack() as ctx:
        consts = ctx.enter_context(tc.tile_pool(name="consts", bufs=1))
        cache = ctx.enter_context(tc.tile_pool(name="cache", bufs=1))
        xs = ctx.enter_context(tc.tile_pool(name="xs", bufs=2))
        x1s = ctx.enter_context(tc.tile_pool(name="x1s", bufs=3))
        tsb = ctx.enter_context(tc.tile_pool(name="tsb", bufs=4))
        ys = ctx.enter_context(tc.tile_pool(name="ys", bufs=2))
        small = ctx.enter_context(tc.tile_pool(name="small", bufs=1))
        dramp = ctx.enter_context(tc.tile_pool(name="dramp", bufs=1, space="DRAM"))
        tps = ctx.enter_context(tc.tile_pool(name="tps", bufs=5, space="PSUM"))
        eps = ctx.enter_context(tc.tile_pool(name="eps", bufs=1, space="PSUM"))
        ops = ctx.enter_context(tc.tile_pool(name="ops", bufs=2, space="PSUM"))

        ident = consts.tile([P, P], F16)
        make_identity(nc, ident)
        ones = consts.tile([1, FM], F16)
        nc.vector.memset(ones[:], 1.0)
        ones_bc = bass.AP(
            tensor=ones.tensor, offset=ones.offset,
            ap=[list(ones.ap[0]), [0, FS // FM], [1, FM]],
        )

        # W template (zeros + ones row) staged to DRAM early, off critical path
        wscr = dramp.tile([PK, P], F16)
        wtmpl = small.tile([PK, P], F16)
        nc.vector.memset(wtmpl[0:P, :], 0.0)
        nc.sync.dma_start(out=wtmpl[P:PK, :], in_=ones[0:1, 0:P])
        nc.sync.dma_start(out=wscr[:], in_=wtmpl[:])
        escr = dramp.tile([P, P], F32)

        E = eps.tile([P, P], F32)
        xh = [cache.tile([PK, FS], F16, name=f"xh{w}", tag=f"xh{w}")
              for w in range(ws)]

        # ~5.6us of dummy matmuls so the PE HAM clock-gate opens before the
        # real pass-A stream arrives (and stays open)
        for _ in range(60):
            wm = ops.tile([P, P], F32, tag="op")
            nc.tensor.matmul(wm[:], lhsT=ident[:], rhs=ident[:],
                             start=True, stop=True)

        # ---------------- pass A: stream x2, cast, transpose, gram ----------
        pend = []          # (tt, idx) awaiting gram matmul
        gi = 0             # gram matmuls emitted

        def emit_gram(tt_ap):
            nonlocal gi
            nc.tensor.matmul(E[:], lhsT=tt_ap, rhs=tt_ap,
                             start=(gi == 0), stop=(gi == n_gram - 1))
            gi += 1

        GRP = 4                       # transpose chunks batched per PSUM bank
        for w in range(ws):
            xt = xs.tile([P, FS], F32)
            dmae = nc.sync if w % 2 == 0 else nc.scalar
            dmae.dma_start(out=xt[:], in_=x2v[:, :, w, :])
            nc.sync.dma_start(out=xh[w][P:PK, :], in_=ones_bc)  # "+1" row
            if w % 2 == 0:
                nc.vector.tensor_copy(out=xh[w][0:P, :], in_=xt[:])  # f32->f16
            else:
                nc.scalar.copy(xh[w][0:P, :], xt[:])
            for g in range(cpt // GRP):
                tp = tps.tile([128, GRP * P], F16)
                for k in range(GRP):
                    c = g * GRP + k
                    nc.tensor.transpose(
                        tp[:, k * P:(k + 1) * P],
                        xh[w][0:P, c * 128:(c + 1) * 128], ident[:])
                tt = tsb.tile([128, GRP * P], F16)
                if (w * (cpt // GRP) + g) % 4 == 0:
                    nc.scalar.copy(tt[:], tp[:])
                else:
                    nc.vector.tensor_copy(out=tt[:], in_=tp[:])
                for k in range(GRP):
                    pend.append(tt[:, k * P:(k + 1) * P])
                while len(pend) > PIPE:
                    emit_gram(pend.pop(0))
        for tt in pend:
            emit_gram(tt)
        pend = []

        # ---------------- energy -> attention -> weights --------------------
        e_sb = small.tile([P, P], F32)
        nc.scalar.copy(e_sb[:], E[:])                  # PSUM -> SBUF
        nc.sync.dma_start(out=escr[:], in_=e_sb[:])    # bounce via DRAM
        eb = small.tile([NN, Q, NN], F32)
        # gather the 16 diagonal 7x7 blocks in one DMA
        diag_src = bass.AP(
            tensor=escr.tensor, offset=escr.offset,
            ap=[[P, NN], [NN * P + NN, Q], [1, NN]],
        )
        nc.sync.dma_start(out=eb[:], in_=diag_src)
        e7 = small.tile([NN, NN], F32)
        nc.vector.tensor_reduce(
            out=e7[:], in_=eb[:].rearrange("p q m -> p m q"),
            axis=mybir.AxisListType.X, op=mybir.AluOpType.add,
        )
        mn = small.tile([NN, 1], F32)
        nc.vector.tensor_reduce(
            out=mn[:], in_=e7[:], axis=mybir.AxisListType.X,
            op=mybir.AluOpType.min,
        )
        ex = small.tile([NN, NN], F32)
        nc.scalar.activation(
            out=ex[:], in_=e7[:], func=mybir.ActivationFunctionType.Exp,
            bias=mn[:], scale=-1.0,
        )                                              # exp(rowmin - E)
        z = small.tile([NN, 1], F32)
        nc.vector.tensor_reduce(
            out=z[:], in_=ex[:], axis=mybir.AxisListType.X,
            op=mybir.AluOpType.add,
        )
        r = small.tile([NN, 1], F32)
        nc.vector.reciprocal(r[:], z[:])
        gsb = small.tile([NN, 1], F32)
        nc.gpsimd.dma_start(
            out=gsb[:],
            in_=bass.AP(tensor=gm[:].tensor, offset=0, ap=[[0, NN], [1, 1]]),
        )
        rg = small.tile([NN, 1], F32)
        nc.vector.tensor_mul(rg[:], r[:], gsb[:])      # gamma / Z_n
        a32 = small.tile([32, 32], F32)
        nc.vector.memset(a32[:], 0.0)
        nc.vector.tensor_scalar_mul(a32[0:NN, 0:NN], ex[:], rg[:])  # gamma*att
        at32 = small.tile([32, 32], F32)
        nc.vector.transpose(at32[:], a32[:])           # (gamma*att)^T
        # scatter the 16 diag blocks of W via DRAM (src bcast over q, f32->f16)
        # dims ordered (m, q, n) on both sides; src partition dim must be first
        diag_dst = bass.AP(
            tensor=wscr.tensor, offset=wscr.offset,
            ap=[[P, NN], [NN * P + NN, Q], [1, NN]],
        )
        at_src = bass.AP(
            tensor=at32.tensor, offset=at32.offset,
            ap=[[at32.ap[0][0], NN], [0, Q], [1, NN]],
        )
        nc.gpsimd.dma_start(out=diag_dst, in_=at_src)  # casts f32 -> f16
        wt = small.tile([PK, P], F16)
        nc.sync.dma_start(out=wt[:], in_=wscr[:])

        # ---------------- pass B: out = W.T @ Xh; y = out * x1 --------------
        for w in range(ws):
            x1t = x1s.tile([P, FS], F32)
            x1e = nc.scalar if w % 2 == 0 else nc.sync
            x1e.dma_start(out=x1t[:], in_=x1v[:, :, w, :])
            yt = ys.tile([P, FS], F32)
            for j in range(mpt):
                sl = slice(j * FM, (j + 1) * FM)
                op = ops.tile([P, FM], F32, tag="op")
                nc.tensor.matmul(op[:], lhsT=wt[:], rhs=xh[w][:, sl],
                                 start=True, stop=True)
                nc.vector.tensor_mul(yt[:, sl], op[:], x1t[:, sl])
            ye = nc.sync if w % 2 == 0 else nc.scalar
            ye.dma_start(out=yv[:, :, w, :], in_=yt[:])

    nc.compile()
    return nc


_NC_CACHE = {}


def _get_nc(d_total=D_FULL):
    if d_total not in _NC_CACHE:
        _NC_CACHE[d_total] = build_nc(d_total)
    return _NC_CACHE[d_total]


def kernel(x1: np.ndarray, x2: np.ndarray, gamma: np.ndarray) -> np.ndarray:
    b, n, c, h, w = x1.shape
    assert (b, n) == (B, NN)
    d = c * h * w
    x1r = np.ascontiguousarray(x1.reshape(b, n, d)).astype(np.float32, copy=False)
    x2r = np.ascontiguousarray(x2.reshape(b, n, d)).astype(np.float32, copy=False)
    g = np.asarray(gamma, dtype=np.float32).reshape(1)

    nc = _get_nc(d)
    in_maps = [
        {"x1": x1r[i], "x2": x2r[i], "gamma": g} for i in range(N_CORES)
    ]
    res = run_bass_kernel_spmd(nc, in_maps, list(range(N_CORES)))
    out = np.stack([res.results[i]["y"] for i in range(N_CORES)], axis=0)
    return out.reshape(b, n, c, h, w).astype(np.float32, copy=False)
